# revision 4
# baseline (speedup 1.0000x reference)
"""CRSDBlock Trainium2 Bass kernel.

Reference computation (2 stacked recurrent layers, T timesteps):
    h' = tanh(x_t @ Wx.T + h @ Wh.T + r1 @ V1.T + r2 @ V2.T + b)
    r1' = 0.9 r1 + 0.1 tanh(h' @ U1.T)
    r2' = 0.9 r2 + 0.1 tanh(h' @ U2.T)
layer output = sequence of h', which feeds the next layer.

Design (single NeuronCore, both layers run serially):
  * The input projection x_t @ Wx.T + b is NOT recurrent -> computed per
    B-step block as a dense matmul into "xpb" (feature-partitioned layout,
    [128 part, B, 8 chunks]).
  * Recurrent state z [128, 14]: cols 0:8 = h (8 chunks of 128), cols 8:14 =
    rho = 10*r (scaled reservoir state).  Scaling 0.1 is baked into the V
    columns of the combined matrix M1 = [Wh | 0.1*V1 | 0.1*V2] so the
    reservoir update is exactly rho' = 0.9*rho + tanh(U @ h').
  * Per step: ACC[128,8] = M1 @ z (112 weight-stationary matmuls accumulated
    in PSUM), h' = tanh(ACC + xpb_t), G[128,6] = U @ h' (48 matmuls),
    rho = 0.9*rho + tanh(G), h' copied into the output block.
  * Steps iterate with a hardware For_i loop; blocks/layers are unrolled in
    python.  Layer 1 writes its h sequence to internal DRAM; layer 2 reads it
    back as its dense-projection input.  All host-side tensors are
    pre-transposed so the device never transposes anything.
"""

import numpy as np

import concourse.bass as bass
import concourse.mybir as mybir
from concourse import bacc, tile
from concourse.bass_utils import run_bass_kernel_spmd

FP32 = mybir.dt.float32
D = 1024          # d_h
DR1, DR2 = 512, 256
DG = DR1 + DR2    # 768 = rows of U = reservoir size
DZ = D + DG       # 1792 = len([h; rho1; rho2])
KC_H = D // 128   # 8  h chunks
KC_Z = DZ // 128  # 14 z chunks
MC_H = D // 128   # 8  ACC output chunks
MC_G = DG // 128  # 6  G output chunks
ALPHA = 0.1


def _tiles128(mat_T, kc, mc):
    """[kc*128, mc*128] pre-transposed matrix -> sbuf layout [128, kc*mc*128]
    where lhsT tile (k,m) = sbuf[:, (k*mc+m)*128 : +128]."""
    return np.ascontiguousarray(
        mat_T.reshape(kc, 128, mc, 128).transpose(1, 0, 2, 3).reshape(128, -1)
    )


def _build(T, B, rdt=FP32, staggered=False):
    nblk = T // B
    nc = bacc.Bacc("TRN2", target_bir_lowering=False, debug=False, num_devices=1)

    m1t_d = nc.dram_tensor("m1t", [2, 128, KC_Z * MC_H * 128], rdt, kind="ExternalInput")
    ut_d = nc.dram_tensor("ut", [2, 128, KC_H * MC_G * 128], rdt, kind="ExternalInput")
    wxt_d = nc.dram_tensor("wxt", [2, 128, KC_H * MC_H * 128], FP32, kind="ExternalInput")
    b_d = nc.dram_tensor("bb", [2, 1, D], FP32, kind="ExternalInput")
    ones_d = nc.dram_tensor("ones", [1, B], FP32, kind="ExternalInput")
    xin_d = nc.dram_tensor("xin", [nblk, 128, B, KC_H], FP32, kind="ExternalInput")
    h1_d = nc.dram_tensor("h1seq", [nblk, 128, B, KC_H], FP32)
    out_d = nc.dram_tensor("out", [nblk, 128, B, KC_H], FP32, kind="ExternalOutput")

    with tile.TileContext(nc) as tc:
        with (
            tc.tile_pool(name="wpool", bufs=1) as wpool,
            tc.tile_pool(name="state", bufs=1) as spool,
            tc.tile_pool(name="blk", bufs=2) as bpool,
            tc.tile_pool(name="ps", bufs=2, space="PSUM") as pspool,
            tc.tile_pool(name="psd", bufs=2, space="PSUM") as psdpool,
        ):
            z = spool.tile([128, KC_Z], rdt, tag="z")
            tg = spool.tile([128, MC_G], rdt, tag="tg")
            hx = spool.tile([128, MC_H], FP32, tag="hx")
            ones_sb = spool.tile([1, B], FP32, tag="ones")
            nc.sync.dma_start(ones_sb[:], ones_d[0])

            for l in range(2):
                m1t = wpool.tile([128, KC_Z * MC_H * 128], rdt, tag="m1t")
                ut = wpool.tile([128, KC_H * MC_G * 128], rdt, tag="ut")
                wxt = wpool.tile([128, KC_H * MC_H * 128], FP32, tag="wxt")
                b_sb = wpool.tile([1, D], FP32, tag="b")
                nc.sync.dma_start(m1t[:], m1t_d[l])
                nc.sync.dma_start(ut[:], ut_d[l])
                nc.sync.dma_start(wxt[:], wxt_d[l])
                nc.sync.dma_start(b_sb[:], b_d[l])
                nc.gpsimd.memset(z[:], 0.0)

                src = xin_d if l == 0 else h1_d
                dst = h1_d if l == 0 else out_d

                for blk in range(nblk):
                    inb = bpool.tile([128, B, KC_H], FP32, tag="inb")
                    xpb = bpool.tile([128, B, MC_H], FP32, tag="xpb")
                    outb = bpool.tile([128, B, KC_H], FP32, tag="outb")
                    nc.sync.dma_start(inb[:], src[blk])

                    # dense phase: xpb[:, t, m] = b[m-chunk] + sum_k WxT_tile(k,m) @ inb[:, t, k]
                    for m in range(MC_H):
                        P = psdpool.tile([128, B], FP32, tag="pdense")
                        nc.tensor.matmul(
                            P[:], b_sb[0:1, m * 128:(m + 1) * 128], ones_sb[0:1, :],
                            start=True, stop=False,
                        )
                        for k in range(KC_H):
                            nc.tensor.matmul(
                                P[:], wxt[:, (k * MC_H + m) * 128:(k * MC_H + m + 1) * 128],
                                inb[:, :, k],
                                start=False, stop=(k == KC_H - 1),
                            )
                        nc.vector.tensor_copy(xpb[:, :, m], P[:])

                    ACC = pspool.tile([128, MC_H], FP32, tag="acc")
                    G = pspool.tile([128, MC_G], FP32, tag="g")

                    with tc.For_i(0, B, 1, staggered_reset=staggered) as i:
                        # ACC = M1 @ z   (z = [h; rho])
                        for m in range(MC_H):
                            for k in range(KC_Z):
                                nc.tensor.matmul(
                                    ACC[:, m:m + 1],
                                    m1t[:, (k * MC_H + m) * 128:(k * MC_H + m + 1) * 128],
                                    z[:, k:k + 1],
                                    start=(k == 0), stop=(k == KC_Z - 1),
                                )
                        # h' = tanh(ACC + xpb_t)  -> z[:, 0:8] and out block
                        nc.vector.tensor_add(hx[:], ACC[:], xpb[:, bass.ds(i, 1), :].opt())
                        nc.scalar.activation(z[:, 0:KC_H], hx[:], mybir.ActivationFunctionType.Tanh)
                        # G = U @ h'
                        for m in range(MC_G):
                            for k in range(KC_H):
                                nc.tensor.matmul(
                                    G[:, m:m + 1],
                                    ut[:, (k * MC_G + m) * 128:(k * MC_G + m + 1) * 128],
                                    z[:, k:k + 1],
                                    start=(k == 0), stop=(k == KC_H - 1),
                                )
                        nc.scalar.activation(tg[:], G[:], mybir.ActivationFunctionType.Tanh)
                        # rho = 0.9*rho + tanh(G)
                        nc.vector.tensor_scalar(
                            z[:, KC_H:KC_Z], z[:, KC_H:KC_Z], 1.0 - ALPHA, None,
                            mybir.AluOpType.mult,
                        )
                        nc.vector.tensor_add(z[:, KC_H:KC_Z], z[:, KC_H:KC_Z], tg[:])
                        nc.vector.tensor_copy(outb[:, bass.ds(i, 1), :].opt(), z[:, 0:KC_H])

                    nc.sync.dma_start(dst[blk], outb[:])

    nc.compile()
    return nc


_CACHE = {}


def _get_nc(T, B, rdt, staggered):
    key = (T, B, rdt, staggered)
    if key not in _CACHE:
        _CACHE[key] = _build(T, B, rdt, staggered)
    return _CACHE[key]


def _prep_inputs(x_seq, Wx, Wh, b, V1, U1, V2, U2, B, rdt=FP32):
    T = x_seq.shape[0]
    nblk = T // B
    f32 = np.float32
    rnp = mybir.dt.np(rdt)
    m1t = np.stack([
        _tiles128(np.concatenate(
            [Wh[l], ALPHA * V1[l], ALPHA * V2[l]], axis=1).T.astype(f32), KC_Z, MC_H)
        for l in range(2)
    ])
    ut = np.stack([
        _tiles128(np.concatenate([U1[l], U2[l]], axis=0).T.astype(f32), KC_H, MC_G)
        for l in range(2)
    ])
    wxt = np.stack([_tiles128(Wx[l].T.astype(f32), KC_H, MC_H) for l in range(2)])
    xin = np.ascontiguousarray(
        x_seq.astype(f32).reshape(nblk, B, KC_H, 128).transpose(0, 3, 1, 2)
    )
    return {
        "m1t": m1t.astype(rnp),
        "ut": ut.astype(rnp),
        "wxt": wxt.astype(f32),
        "bb": np.ascontiguousarray(b.astype(f32).reshape(2, 1, D)),
        "ones": np.ones((1, B), f32),
        "xin": xin,
    }


def run_kernel(x_seq, Wx, Wh, b, V1, U1, V2, U2, B=256, trace=False, rdt=FP32, staggered=False):
    T = x_seq.shape[0]
    nc = _get_nc(T, B, rdt, staggered)
    in_map = _prep_inputs(x_seq, Wx, Wh, b, V1, U1, V2, U2, B, rdt)
    res = run_bass_kernel_spmd(nc, [in_map], core_ids=[0], trace=trace)
    out = res.results[0]["out"]  # [nblk, 128, B, 8]
    h2 = out.transpose(0, 2, 3, 1).reshape(T, D)
    return np.ascontiguousarray(h2), res


def kernel(x_seq, Wx, Wh, b, V1, U1, V2, U2):
    h2, _ = run_kernel(
        np.asarray(x_seq), np.asarray(Wx), np.asarray(Wh), np.asarray(b),
        np.asarray(V1), np.asarray(U1), np.asarray(V2), np.asarray(U2),
    )
    return h2


# revision 5
# speedup vs baseline: 1.4767x; 1.4767x over previous
"""CRSDBlock Trainium2 Bass kernel.

Reference computation (2 stacked recurrent layers, T timesteps):
    h' = tanh(x_t @ Wx.T + h @ Wh.T + r1 @ V1.T + r2 @ V2.T + b)
    r1' = 0.9 r1 + 0.1 tanh(h' @ U1.T)
    r2' = 0.9 r2 + 0.1 tanh(h' @ U2.T)
layer output = sequence of h', which feeds the next layer.

Design (single NeuronCore, both layers run serially):
  * The input projection x_t @ Wx.T + b is NOT recurrent -> computed per
    B-step block as a dense matmul into "xpb" (feature-partitioned layout,
    [128 part, B, 8 chunks]).
  * Recurrent state z [128, 14]: cols 0:8 = h (8 chunks of 128), cols 8:14 =
    rho = 10*r (scaled reservoir state).  Scaling 0.1 is baked into the V
    columns of the combined matrix M1 = [Wh | 0.1*V1 | 0.1*V2] so the
    reservoir update is exactly rho' = 0.9*rho + tanh(U @ h').
  * Per step: ACC[128,8] = M1 @ z (112 weight-stationary matmuls accumulated
    in PSUM), h' = tanh(ACC + xpb_t), G[128,6] = U @ h' (48 matmuls),
    rho = 0.9*rho + tanh(G), h' copied into the output block.
  * Steps iterate with a hardware For_i loop; blocks/layers are unrolled in
    python.  Layer 1 writes its h sequence to internal DRAM; layer 2 reads it
    back as its dense-projection input.  All host-side tensors are
    pre-transposed so the device never transposes anything.
"""

import numpy as np

import concourse.bass as bass
import concourse.mybir as mybir
from concourse import bacc, tile
from concourse.bass_utils import run_bass_kernel_spmd

FP32 = mybir.dt.float32
D = 1024          # d_h
DR1, DR2 = 512, 256
DG = DR1 + DR2    # 768 = rows of U = reservoir size
DZ = D + DG       # 1792 = len([h; rho1; rho2])
KC_H = D // 128   # 8  h chunks
KC_Z = DZ // 128  # 14 z chunks
MC_H = D // 128   # 8  ACC output chunks
MC_G = DG // 128  # 6  G output chunks
ALPHA = 0.1


def _tiles128(mat_T, kc, mc):
    """[kc*128, mc*128] pre-transposed matrix -> sbuf layout [128, kc*mc*128]
    where lhsT tile (k,m) = sbuf[:, (k*mc+m)*128 : +128]."""
    return np.ascontiguousarray(
        mat_T.reshape(kc, 128, mc, 128).transpose(1, 0, 2, 3).reshape(128, -1)
    )


def _build(T, B, rdt=FP32, staggered=False):
    nblk = T // B
    nc = bacc.Bacc("TRN2", target_bir_lowering=False, debug=False, num_devices=1)

    m1t_d = nc.dram_tensor("m1t", [2, 128, KC_Z * MC_H * 128], rdt, kind="ExternalInput")
    ut_d = nc.dram_tensor("ut", [2, 128, KC_H * MC_G * 128], rdt, kind="ExternalInput")
    wxt_d = nc.dram_tensor("wxt", [2, 128, KC_H * MC_H * 128], FP32, kind="ExternalInput")
    b_d = nc.dram_tensor("bb", [2, 1, D], FP32, kind="ExternalInput")
    ones_d = nc.dram_tensor("ones", [1, B], FP32, kind="ExternalInput")
    xin_d = nc.dram_tensor("xin", [nblk, 128, B, KC_H], FP32, kind="ExternalInput")
    h1_d = nc.dram_tensor("h1seq", [nblk, 128, B, KC_H], FP32)
    out_d = nc.dram_tensor("out", [nblk, 128, B, KC_H], FP32, kind="ExternalOutput")

    with tile.TileContext(nc) as tc:
        with (
            tc.tile_pool(name="wpool", bufs=1) as wpool,
            tc.tile_pool(name="state", bufs=1) as spool,
            tc.tile_pool(name="blk", bufs=2) as bpool,
            tc.tile_pool(name="ps", bufs=2, space="PSUM") as pspool,
            tc.tile_pool(name="psd", bufs=2, space="PSUM") as psdpool,
        ):
            z = spool.tile([128, KC_Z], rdt, tag="z")
            tg = spool.tile([128, MC_G], rdt, tag="tg")
            hx = spool.tile([128, MC_H], FP32, tag="hx")
            ones_sb = spool.tile([1, B], FP32, tag="ones")
            nc.sync.dma_start(ones_sb[:], ones_d[0])

            for l in range(2):
                m1t = wpool.tile([128, KC_Z * MC_H * 128], rdt, tag="m1t")
                ut = wpool.tile([128, KC_H * MC_G * 128], rdt, tag="ut")
                wxt = wpool.tile([128, KC_H * MC_H * 128], FP32, tag="wxt")
                b_sb = wpool.tile([1, D], FP32, tag="b")
                nc.sync.dma_start(m1t[:], m1t_d[l])
                nc.sync.dma_start(ut[:], ut_d[l])
                nc.sync.dma_start(wxt[:], wxt_d[l])
                nc.sync.dma_start(b_sb[:], b_d[l])
                nc.gpsimd.memset(z[:], 0.0)

                src = xin_d if l == 0 else h1_d
                dst = h1_d if l == 0 else out_d

                inb = bpool.tile([128, B, KC_H], FP32, tag="inb")
                xpb = bpool.tile([128, B, MC_H], FP32, tag="xpb")
                outb = bpool.tile([128, B, KC_H], FP32, tag="outb")
                ACC = pspool.tile([128, MC_H], FP32, tag="acc")
                G = pspool.tile([128, MC_G], FP32, tag="g")

                with tc.For_i(0, nblk, 1) as bv:
                    nc.sync.dma_start(inb[:], src[bass.ds(bv, 1)].opt())

                    # dense phase: xpb[:, t, m] = b[m-chunk] + sum_k WxT_tile(k,m) @ inb[:, t, k]
                    for m in range(MC_H):
                        P = psdpool.tile([128, B], FP32, tag="pdense")
                        nc.tensor.matmul(
                            P[:], b_sb[0:1, m * 128:(m + 1) * 128], ones_sb[0:1, :],
                            start=True, stop=False,
                        )
                        for k in range(KC_H):
                            nc.tensor.matmul(
                                P[:], wxt[:, (k * MC_H + m) * 128:(k * MC_H + m + 1) * 128],
                                inb[:, :, k],
                                start=False, stop=(k == KC_H - 1),
                            )
                        nc.vector.tensor_copy(xpb[:, :, m], P[:])

                    with tc.For_i(0, B, 1, staggered_reset=staggered) as i:
                        # ACC = M1 @ z   (z = [h; rho])
                        for m in range(MC_H):
                            for k in range(KC_Z):
                                nc.tensor.matmul(
                                    ACC[:, m:m + 1],
                                    m1t[:, (k * MC_H + m) * 128:(k * MC_H + m + 1) * 128],
                                    z[:, k:k + 1],
                                    start=(k == 0), stop=(k == KC_Z - 1),
                                )
                        # h' = tanh(ACC + xpb_t)  -> z[:, 0:8] and out block
                        nc.vector.tensor_add(hx[:], ACC[:], xpb[:, bass.ds(i, 1), :].opt())
                        nc.scalar.activation(z[:, 0:KC_H], hx[:], mybir.ActivationFunctionType.Tanh)
                        # G = U @ h'
                        for m in range(MC_G):
                            for k in range(KC_H):
                                nc.tensor.matmul(
                                    G[:, m:m + 1],
                                    ut[:, (k * MC_G + m) * 128:(k * MC_G + m + 1) * 128],
                                    z[:, k:k + 1],
                                    start=(k == 0), stop=(k == KC_H - 1),
                                )
                        nc.scalar.activation(tg[:], G[:], mybir.ActivationFunctionType.Tanh)
                        # rho = 0.9*rho + tanh(G)
                        nc.vector.tensor_scalar(
                            z[:, KC_H:KC_Z], z[:, KC_H:KC_Z], 1.0 - ALPHA, None,
                            mybir.AluOpType.mult,
                        )
                        nc.vector.tensor_add(z[:, KC_H:KC_Z], z[:, KC_H:KC_Z], tg[:])
                        nc.vector.tensor_copy(outb[:, bass.ds(i, 1), :].opt(), z[:, 0:KC_H])

                    nc.sync.dma_start(dst[bass.ds(bv, 1)].opt(), outb[:])

    nc.compile()
    return nc


_CACHE = {}


def _get_nc(T, B, rdt, staggered):
    key = (T, B, rdt, staggered)
    if key not in _CACHE:
        _CACHE[key] = _build(T, B, rdt, staggered)
    return _CACHE[key]


def _prep_inputs(x_seq, Wx, Wh, b, V1, U1, V2, U2, B, rdt=FP32):
    T = x_seq.shape[0]
    nblk = T // B
    f32 = np.float32
    rnp = mybir.dt.np(rdt)
    m1t = np.stack([
        _tiles128(np.concatenate(
            [Wh[l], ALPHA * V1[l], ALPHA * V2[l]], axis=1).T.astype(f32), KC_Z, MC_H)
        for l in range(2)
    ])
    ut = np.stack([
        _tiles128(np.concatenate([U1[l], U2[l]], axis=0).T.astype(f32), KC_H, MC_G)
        for l in range(2)
    ])
    wxt = np.stack([_tiles128(Wx[l].T.astype(f32), KC_H, MC_H) for l in range(2)])
    xin = np.ascontiguousarray(
        x_seq.astype(f32).reshape(nblk, B, KC_H, 128).transpose(0, 3, 1, 2)
    )
    return {
        "m1t": m1t.astype(rnp),
        "ut": ut.astype(rnp),
        "wxt": wxt.astype(f32),
        "bb": np.ascontiguousarray(b.astype(f32).reshape(2, 1, D)),
        "ones": np.ones((1, B), f32),
        "xin": xin,
    }


def run_kernel(x_seq, Wx, Wh, b, V1, U1, V2, U2, B=256, trace=False, rdt=FP32, staggered=False):
    T = x_seq.shape[0]
    nc = _get_nc(T, B, rdt, staggered)
    in_map = _prep_inputs(x_seq, Wx, Wh, b, V1, U1, V2, U2, B, rdt)
    res = run_bass_kernel_spmd(nc, [in_map], core_ids=[0], trace=trace)
    out = res.results[0]["out"]  # [nblk, 128, B, 8]
    h2 = out.transpose(0, 2, 3, 1).reshape(T, D)
    return np.ascontiguousarray(h2), res


def kernel(x_seq, Wx, Wh, b, V1, U1, V2, U2):
    h2, _ = run_kernel(
        np.asarray(x_seq), np.asarray(Wx), np.asarray(Wh), np.asarray(b),
        np.asarray(V1), np.asarray(U1), np.asarray(V2), np.asarray(U2),
    )
    return h2
